# revision 4
# baseline (speedup 1.0000x reference)
"""Trainium2 Bass kernel for windowed multi-head attention (nn_AttentionWindow).

Reference computation (B=64, N=197, DIM=768, H=12, HD=64):
    qkv  = x @ qkv_w.T + [q_bias, 0, v_bias]
    q, k, v = split(qkv);  q *= HD**-0.5
    attn = softmax(q @ k.T + rpb_table[rel_index])
    out  = (attn @ v) @ proj_w.T + proj_b
Sharding: data-parallel over batch across 8 NeuronCores (8 batches/core).

Per-core design (bf16 matmuls on TensorE, fp32 PSUM accumulation):
  - Ramp: input DMA descriptors are issued from THREE engines in
    parallel (Scalar, Sync, GpSimd; ~650ns serial issue cost each), x
    is split per (kc, token-slice) and qkv_w per 128-col output chunk
    so the first qk matmul group's inputs land ASAP; dummy warm-up
    matmuls on a memset tile cover the DMA window so the HAM clock
    gate is released (2.4 GHz) before real work starts.
  - q,k computed feature-major (slice-outer, chunk-inner to match DMA
    arrival) into one resident [128, 12, 1576] bf16 tile.
  - v: token-major per batch ([tokens, channels], 128+69 row chunks) so
    the attention contraction has tokens on partitions; stored bf16 in
    one resident tile. The first SKEW attention stage_a's are emitted
    between the last v groups (pipeline pre-fill).
  - Scores transposed: S^T[j,i] = k_h[:,j]^T q_h, per (head-pair,
    batch) item; the two heads of a pair use opposite 64-row halves of
    the PE array (row-packing -> concurrent matmuls). Softmax WITHOUT
    max subtraction (scores are O(1): q pre-scaled by 1/8): ONE exp per
    item on ScalarE covering all 4 score blocks (j1/j2 x h0/h1) via a
    strided AP (j2's partitions 69:128 compute garbage that is never
    read) -> P^T bf16; bias multiply softmax(S+B) = expS*expB / sums
    is split: j1 blocks on VectorE, j2 blocks on GpSimd (engine
    balance - DVE would otherwise be the attention bottleneck).
  - O^T[d,i] = sum_j v[j,d] P^T[j,i]: head pair col-packed via
    tile_position (0,0)/(0,64); denominators via 64-col ones-matmuls
    col-packed the same way into the SAME PSUM bank as O^T (cols
    256:453) -> ONE reciprocal_approx_fast + ONE normalize multiply
    [128,197] per item on VectorE, fused into the PSUM->SBUF copy
    assembling proj's rhs (resident [128, 6, 1576]). j1 matmuls (which
    only need the VectorE bias product) issue before j2 ones (GpSimd).
  - Attention software-pipelined (skew 3); projection for batch b
    (feature-major, free dim = that batch's 197 tokens) is interleaved
    into the attention stream TWO items after b's last stage_b (so
    TensorE never waits on the recip+normalize latency), keeping the
    PE warm end-to-end and hiding the proj phase + output DMA.
"""
import sys
import functools

sys.path.insert(0, "/opt/trn_rl_repo")

import numpy as np
import ml_dtypes

import concourse.bass as bass  # noqa: E402
import concourse.bacc as bacc  # noqa: E402
import concourse.mybir as mybir  # noqa: E402
from concourse.tile import TileContext  # noqa: E402
from concourse.bass_utils import run_bass_kernel_spmd  # noqa: E402

F32 = mybir.dt.float32
BF16 = mybir.dt.bfloat16

NCORES = 8
B, NT, DIM = 64, 197, 768
H, HD = 12, 64
SCALE = HD ** -0.5  # 0.125, exact power of two -> folded into q weights
BPC = B // NCORES   # 8 batches per core
TOK = BPC * NT      # 1576 tokens per core
KC = DIM // 128     # 6
NT2 = NT - 128      # 69 (second token chunk)
SKEW = 3            # attention software-pipeline depth (items)
N_WARM = 16         # dummy warm-up matmuls issued under the input DMAs
# 512-token slices for the token-parallel qk matmuls
SLICES = [(s * 512, min(TOK, (s + 1) * 512)) for s in range((TOK + 511) // 512)]


def build(qkv_bias_nonzero: bool, proj_bias_nonzero: bool):
    nc = bacc.Bacc("TRN2", target_bir_lowering=False, debug=False)

    xt = nc.dram_tensor("xt", [DIM, TOK], BF16, kind="ExternalInput")
    # qkv_w for q,k in per-output-chunk format [p, c, kc, x]
    qkw = nc.dram_tensor("qkw", [128, 12 * KC * 128], BF16, kind="ExternalInput")
    # v / proj weights in [p, kc, x] format
    vw = nc.dram_tensor("vw", [128, KC * DIM], BF16, kind="ExternalInput")
    pw = nc.dram_tensor("pw", [128, KC * DIM], BF16, kind="ExternalInput")
    ebm = nc.dram_tensor("ebm", [128, KC * 4 * NT], BF16, kind="ExternalInput")
    out = nc.dram_tensor("out", [DIM, TOK], F32, kind="ExternalOutput")
    if qkv_bias_nonzero:
        qkb = nc.dram_tensor("qkb", [1, 2 * DIM], BF16, kind="ExternalInput")
        vb = nc.dram_tensor("vb", [1, DIM], BF16, kind="ExternalInput")
    if proj_bias_nonzero:
        pb = nc.dram_tensor("pb", [1, DIM], BF16, kind="ExternalInput")

    with TileContext(nc) as tc:
        with (
            tc.tile_pool(name="const", bufs=1) as constp,
            tc.tile_pool(name="pp", bufs=SKEW + 2) as pp,
            tc.tile_pool(name="rcp", bufs=3) as rcp,
            tc.tile_pool(name="obp", bufs=4) as obp,
            tc.tile_pool(name="ps", bufs=4, space="PSUM") as ps,
            tc.tile_pool(name="sta", bufs=2, space="PSUM") as sta,
        ):
            # ---- warm-up scratch (no DMA dependency) ----
            ones_bf = constp.tile([128, 128], BF16, name="ones_bf")
            nc.gpsimd.memset(ones_bf[:, :], 1.0)
            scr = constp.tile([128, 512], BF16, name="scr")
            nc.vector.memset(scr[:, :], 0.0)

            # ---- resident tiles ----
            xb_s = constp.tile([128, KC, TOK], BF16, name="xb_s")
            qkw_s = constp.tile([128, KC, 2 * DIM], BF16, name="qkw_s")
            vw_s = constp.tile([128, KC, DIM], BF16, name="vw_s")
            pw_s = constp.tile([128, KC, DIM], BF16, name="pw_s")
            ebm_s = constp.tile([128, KC, 2, 2, NT], BF16, name="ebm_s")
            qk_s = constp.tile([128, 2 * KC, TOK], BF16, name="qk_s")
            op_s = constp.tile([128, KC, TOK], BF16, name="op_s")
            vts = constp.tile([128, 2 * BPC, DIM], BF16, name="vts")

            # ---- input DMAs, issued from 3 engines in parallel ----
            def dma_qkw(eng, c):
                eng.dma_start(qkw_s[:, :, c * 128:(c + 1) * 128],
                              qkw[:, c * KC * 128:(c + 1) * KC * 128])

            def dma_xt(eng, kc, s):
                t0, t1 = SLICES[s]
                eng.dma_start(xb_s[:, kc, t0:t1],
                              xt[kc * 128:(kc + 1) * 128, t0:t1])

            # Scalar: first qk group's inputs first
            dma_qkw(nc.scalar, 0)
            for kc in range(KC):
                dma_xt(nc.scalar, kc, 0)
            dma_qkw(nc.scalar, 1)
            dma_qkw(nc.scalar, 2)
            # Sync: the rest of xt + qkw
            for kc in range(KC):
                dma_xt(nc.sync, kc, 1)
            for c in range(3, 6):
                dma_qkw(nc.sync, c)
            for kc in range(KC):
                dma_xt(nc.sync, kc, 2)
            for c in range(6, 9):
                dma_qkw(nc.sync, c)
            for kc in range(KC):
                dma_xt(nc.sync, kc, 3)
            for c in range(9, 12):
                dma_qkw(nc.sync, c)
            # GpSimd: v/proj weights + bias table (single 3D descriptors)
            nc.gpsimd.dma_start(vw_s[:, :, :], vw[:, :])
            nc.gpsimd.dma_start(
                ebm_s[:, :, :, :, :].rearrange("p a b c x -> p (a b c x)"),
                ebm[:, :])
            nc.gpsimd.dma_start(pw_s[:, :, :], pw[:, :])
            if qkv_bias_nonzero:
                qkb_s = constp.tile([1, 2 * DIM], BF16, name="qkb_s")
                vb_s = constp.tile([1, DIM], BF16, name="vb_s")
                nc.sync.dma_start(qkb_s[:, :], qkb[:, :])
                nc.sync.dma_start(vb_s[:, :], vb[:, :])
            if proj_bias_nonzero:
                pb_s = constp.tile([1, DIM], BF16, name="pb_s")
                nc.sync.dma_start(pb_s[:, :], pb[:, :])
            if qkv_bias_nonzero or proj_bias_nonzero:
                ones_bfr = constp.tile([1, 512], BF16, name="ones_bfr")
                nc.gpsimd.memset(ones_bfr[:, :], 1.0)

            # ---- HAM warm-up under the DMAs ----
            for i in range(N_WARM):
                acc = ps.tile([128, 512], F32, name="acc_w", tag="mm")
                nc.tensor.matmul(acc[:, :], ones_bf[:, :], scr[:, :],
                                 start=True, stop=True)

            # ---- q,k feature-major: slice-outer (DMA arrival order) ----
            for s, (t0, t1) in enumerate(SLICES):
                w = t1 - t0
                for c in range(2 * KC):
                    acc = ps.tile([128, 512], F32, name="acc_qk", tag="mm")
                    for kc in range(KC):
                        nc.tensor.matmul(
                            acc[:, 0:w],
                            qkw_s[:, kc, c * 128:(c + 1) * 128],
                            xb_s[:, kc, t0:t1],
                            start=(kc == 0),
                            stop=(kc == KC - 1) and not qkv_bias_nonzero,
                        )
                    if qkv_bias_nonzero:
                        nc.tensor.matmul(
                            acc[:, 0:w],
                            qkb_s[0:1, c * 128:(c + 1) * 128],
                            ones_bfr[0:1, 0:w],
                            start=False, stop=True,
                        )
                    nc.scalar.copy(qk_s[:, c, t0:t1], acc[:, 0:w])

            # ---- attention stages ----
            def stage_a(b, hp):
                """Scores for heads 2hp,2hp+1 (array row-packed), ONE merged
                exp over all 4 blocks, bias-mult split V(j1)/G(j2) -> P^T."""
                st = sta.tile([128, 1024], F32, name="st", tag="sta")
                q0 = qk_s[0:64, hp, b * NT:(b + 1) * NT]
                q1 = qk_s[64:128, hp, b * NT:(b + 1) * NT]
                nc.tensor.matmul(st[:, 0:NT],
                                 qk_s[0:64, KC + hp, b * NT:b * NT + 128],
                                 q0, start=True, stop=True)
                nc.tensor.matmul(st[:, 512:512 + NT],
                                 qk_s[64:128, KC + hp, b * NT:b * NT + 128],
                                 q1, start=True, stop=True)
                nc.tensor.matmul(st[0:NT2, 256:256 + NT],
                                 qk_s[0:64, KC + hp, b * NT + 128:(b + 1) * NT],
                                 q0, start=True, stop=True)
                nc.tensor.matmul(st[0:NT2, 768:768 + NT],
                                 qk_s[64:128, KC + hp, b * NT + 128:(b + 1) * NT],
                                 q1, start=True, stop=True)
                # ONE exp over [p, h(2), j(2), 197]; rows 69:128 of the j2
                # blocks are garbage lanes (never read downstream)
                pj = pp.tile([128, 2, 2, NT], BF16, name="pj", tag="p")
                stv = st[:, :].rearrange("p (h j x) -> p h j x", h=2, j=2)
                nc.scalar.activation(pj[:, :, :, :], stv[:, :, :, 0:NT],
                                     mybir.ActivationFunctionType.Exp)
                # bias multiply: j1 blocks on VectorE, j2 blocks on GpSimd
                nc.vector.tensor_mul(pj[:, :, 0, :], pj[:, :, 0, :],
                                     ebm_s[:, hp, :, 0, :])
                nc.gpsimd.tensor_mul(pj[0:NT2, :, 1, :], pj[0:NT2, :, 1, :],
                                     ebm_s[0:NT2, hp, :, 1, :])
                return pj

            def stage_b(b, hp, pj):
                """O^T (head pair col-packed) + col-packed 64-wide sums into
                the same PSUM bank + ONE recip + ONE normalize mult.
                j1 matmuls (VectorE product) issue before j2 (GpSimd)."""
                h0, h1 = 2 * hp, 2 * hp + 1
                bb = ps.tile([128, 512], F32, name="bb", tag="mm")
                v0 = vts[:, 2 * b, :]
                v1 = vts[:, 2 * b + 1, :]
                nc.tensor.matmul(
                    bb[0:64, 0:NT], v0[:, h0 * HD:(h0 + 1) * HD],
                    pj[:, 0, 0, :], start=True, stop=False)
                nc.tensor.matmul(
                    bb[64:128, 0:NT], v0[:, h1 * HD:(h1 + 1) * HD],
                    pj[:, 1, 0, :], start=True, stop=False,
                    tile_position=(0, 64))
                nc.tensor.matmul(
                    bb[0:64, 0:NT], v1[0:NT2, h0 * HD:(h0 + 1) * HD],
                    pj[0:NT2, 0, 1, :], start=False, stop=True)
                nc.tensor.matmul(
                    bb[64:128, 0:NT], v1[0:NT2, h1 * HD:(h1 + 1) * HD],
                    pj[0:NT2, 1, 1, :], start=False, stop=True,
                    tile_position=(0, 64))
                nc.tensor.matmul(bb[0:64, 256:256 + NT], ones_bf[:, 0:64],
                                 pj[:, 0, 0, :], start=True, stop=False)
                nc.tensor.matmul(bb[64:128, 256:256 + NT], ones_bf[:, 0:64],
                                 pj[:, 1, 0, :], start=True, stop=False,
                                 tile_position=(0, 64))
                nc.tensor.matmul(bb[0:64, 256:256 + NT], ones_bf[0:NT2, 0:64],
                                 pj[0:NT2, 0, 1, :], start=False, stop=True)
                nc.tensor.matmul(bb[64:128, 256:256 + NT], ones_bf[0:NT2, 0:64],
                                 pj[0:NT2, 1, 1, :], start=False, stop=True,
                                 tile_position=(0, 64))
                rc = rcp.tile([128, NT], F32, name="rc", tag="rc")
                nc.vector.reciprocal_approx_fast(
                    out=rc[:, :], in_=bb[:, 256:256 + NT])
                nc.vector.tensor_mul(
                    op_s[:, hp, b * NT:(b + 1) * NT],
                    bb[:, 0:NT], rc[:, :])

            def proj_batch(b):
                """Projection for batch b: 6 output chunks, free dim = b's
                197 tokens; interleaved into the attention stream."""
                for c in range(KC):
                    acc = ps.tile([128, 512], F32, name="acc_p", tag="mm")
                    for kp in range(KC):
                        nc.tensor.matmul(
                            acc[:, 0:NT],
                            pw_s[:, kp, c * 128:(c + 1) * 128],
                            op_s[:, kp, b * NT:(b + 1) * NT],
                            start=(kp == 0),
                            stop=(kp == KC - 1) and not proj_bias_nonzero,
                        )
                    if proj_bias_nonzero:
                        nc.tensor.matmul(
                            acc[:, 0:NT],
                            pb_s[0:1, c * 128:(c + 1) * 128],
                            ones_bfr[0:1, 0:NT],
                            start=False, stop=True,
                        )
                    obt = obp.tile([128, NT], F32, name="obt", tag="ob")
                    nc.scalar.copy(obt[:, :], acc[:, 0:NT])
                    nc.sync.dma_start(
                        out[c * 128:(c + 1) * 128, b * NT:(b + 1) * NT],
                        obt[:, :])

            items = [(hp, b) for b in range(BPC) for hp in range(KC)]
            pend = {}

            # ---- v token-major per batch, with stage_a pre-fill ----
            for b in range(BPC):
                for tch in range(2):
                    toff = b * NT + tch * 128
                    tlen = 128 if tch == 0 else NT2
                    for half in range(2):
                        n0, n1 = half * 384, (half + 1) * 384
                        acc = ps.tile([128, 512], F32, name="acc_v", tag="mm")
                        for kc in range(KC):
                            nc.tensor.matmul(
                                acc[0:tlen, 0:384],
                                xb_s[:, kc, toff:toff + tlen],
                                vw_s[:, kc, n0:n1],
                                start=(kc == 0),
                                stop=(kc == KC - 1) and not qkv_bias_nonzero,
                            )
                        if qkv_bias_nonzero:
                            nc.tensor.matmul(
                                acc[0:tlen, 0:384],
                                ones_bfr[0:1, 0:tlen],
                                vb_s[0:1, n0:n1],
                                start=False, stop=True,
                            )
                        nc.vector.tensor_copy(
                            vts[0:tlen, 2 * b + tch, n0:n1], acc[0:tlen, 0:384])
                if b >= BPC - SKEW:          # pipeline pre-fill
                    i = b - (BPC - SKEW)
                    hp_i, b_i = items[i]
                    pend[i] = (b_i, hp_i, stage_a(b_i, hp_i))

            # ---- attention + interleaved projection ----
            for j in range(len(items)):
                i = j + SKEW
                if i < len(items):
                    hp_i, b_i = items[i]
                    pend[i] = (b_i, hp_i, stage_a(b_i, hp_i))
                stage_b(*pend.pop(j))
                jj = j - 2                    # delayed proj emission
                if jj >= 0 and items[jj][0] == KC - 1:
                    proj_batch(items[jj][1])
            for jj in range(len(items) - 2, len(items)):
                if items[jj][0] == KC - 1:
                    proj_batch(items[jj][1])

    nc.compile()
    return nc


@functools.lru_cache(maxsize=4)
def _built(qkv_bias_nonzero: bool, proj_bias_nonzero: bool):
    return build(qkv_bias_nonzero, proj_bias_nonzero)


def prepare_inputs(x, qkv_w, q_bias, v_bias, rpb_table, proj_w, proj_b, rel_index):
    """Host-side prep: shard + transpose + fold scale + gather bias table."""
    x = np.asarray(x, dtype=np.float32)
    qkv_w = np.asarray(qkv_w, dtype=np.float32)
    q_bias = np.asarray(q_bias, dtype=np.float32)
    v_bias = np.asarray(v_bias, dtype=np.float32)
    rpb_table = np.asarray(rpb_table, dtype=np.float32)
    proj_w = np.asarray(proj_w, dtype=np.float32)
    proj_b = np.asarray(proj_b, dtype=np.float32)
    rel_index = np.asarray(rel_index)

    qw = qkv_w[0:DIM] * np.float32(SCALE)   # exact: SCALE is a power of two
    qkw_fm = np.ascontiguousarray(
        np.concatenate([qw, qkv_w[DIM:2 * DIM]], axis=0).T)      # [768, 1536]
    # -> [p, c, kc, x] chunk format
    qkw_h = np.ascontiguousarray(
        qkw_fm.reshape(KC, 128, 12, 128).transpose(1, 2, 0, 3)
        .reshape(128, 12 * KC * 128)).astype(ml_dtypes.bfloat16)
    vw_h = np.ascontiguousarray(
        qkv_w[2 * DIM:3 * DIM].T.reshape(KC, 128, DIM).transpose(1, 0, 2)
        .reshape(128, KC * DIM)).astype(ml_dtypes.bfloat16)      # [p, kc, x]
    pw_h = np.ascontiguousarray(
        proj_w.T.reshape(KC, 128, DIM).transpose(1, 0, 2)
        .reshape(128, KC * DIM)).astype(ml_dtypes.bfloat16)

    # bias[i, j, h] -> exp -> ebT[h, j, i] -> merged [p, hp, h, jc, i]
    bias = rpb_table[rel_index]                                  # (197,197,12)
    ebT = np.exp(bias.astype(np.float32)).transpose(2, 1, 0)     # (12, j, i)
    t = ebT.reshape(KC, 2, NT, NT)                               # [hp, h, j, i]
    ebm_h = np.ones((128, KC, 2, 2, NT), np.float32)
    ebm_h[:, :, :, 0, :] = t[:, :, 0:128, :].transpose(2, 0, 1, 3)
    ebm_h[0:NT2, :, :, 1, :] = t[:, :, 128:NT, :].transpose(2, 0, 1, 3)
    ebm_h = np.ascontiguousarray(
        ebm_h.reshape(128, KC * 4 * NT)).astype(ml_dtypes.bfloat16)

    qkv_bias_nonzero = bool(q_bias.any() or v_bias.any())
    proj_bias_nonzero = bool(proj_b.any())

    in_maps = []
    for i in range(NCORES):
        xs = x[i * BPC:(i + 1) * BPC].reshape(TOK, DIM)
        m = {
            "xt": np.ascontiguousarray(xs.T).astype(ml_dtypes.bfloat16),
            "qkw": qkw_h, "vw": vw_h, "pw": pw_h,
            "ebm": ebm_h,
        }
        if qkv_bias_nonzero:
            m["qkb"] = np.ascontiguousarray(
                np.concatenate([q_bias * np.float32(SCALE),
                                np.zeros_like(q_bias)])[None, :],
                dtype=np.float32).astype(ml_dtypes.bfloat16)
            m["vb"] = np.ascontiguousarray(
                v_bias[None, :]).astype(ml_dtypes.bfloat16)
        if proj_bias_nonzero:
            m["pb"] = np.ascontiguousarray(
                proj_b[None, :], dtype=np.float32).astype(ml_dtypes.bfloat16)
        in_maps.append(m)
    return in_maps, qkv_bias_nonzero, proj_bias_nonzero


def kernel(x, qkv_w, q_bias, v_bias, rpb_table, proj_w, proj_b, rel_index):
    in_maps, qb_nz, pb_nz = prepare_inputs(
        x, qkv_w, q_bias, v_bias, rpb_table, proj_w, proj_b, rel_index)
    nc = _built(qb_nz, pb_nz)
    res = run_bass_kernel_spmd(nc, in_maps, core_ids=list(range(NCORES)))
    outs = []
    for i in range(NCORES):
        ofm = res.results[i]["out"]                  # [768, 1576]
        outs.append(ofm.T.reshape(BPC, NT, DIM))
    return np.concatenate(outs, axis=0).astype(np.float32)
